# revision 31
# baseline (speedup 1.0000x reference)
"""MoE layer (N=16384, H=1024, E=8, top-2) on 8 TRN2 NeuronCores.

Two-phase sparse expert-parallel strategy:
  Phase A (gate): data-parallel over tokens (2048/core). fp32 gate matmul on
    PE + exact softmax -> gate_probs (bit-exact vs f32 reference on this HW).
  Host routing: top-2 selection from the returned probs (pure argsort-free
    masking, matches reference semantics), per-expert token gather, pad to
    capacity C, transpose to feature-major, cast to bf16.
  Phase B (FFN): expert-parallel - core e runs expert e's residual block over
    its C gathered tokens. Feature-major [H, tok] layout; LayerNorm over H via
    ones-matmul on PE; per-token stats replicated across partitions by PE
    outer products; combine weight applied on device; host scatter-adds the
    two contributions per token.
  Tokens beyond capacity (never expected for this distribution) are computed
  exactly on the host in f32.
"""

import sys

for _p in ("/opt/trn_rl_repo", "/root/.axon_site/_ro/trn_rl_repo"):
    if _p not in sys.path:
        sys.path.append(_p)

from contextlib import ExitStack

import ml_dtypes
import numpy as np

import concourse.bass as bass
import concourse.tile as tile
from concourse import bacc, mybir
from concourse.bass_interp import get_hw_module
from concourse.bass_utils import run_bass_kernel_spmd

F32 = mybir.dt.float32
BF16 = mybir.dt.bfloat16
AF = mybir.ActivationFunctionType
OP = mybir.AluOpType

N_CORES = 8
N, H, E = 16384, 1024, 8
NT = N // N_CORES          # tokens per core in gate phase (2048)
C = 4608                   # per-expert token capacity in FFN phase
TS = 512                   # token chunk (matmul free dim)
KT = H // 128              # 8 k/m tiles
LN_EPS = 1e-5

_CACHE = {}


def _build_gate():
    nc = bacc.Bacc("TRN2", target_bir_lowering=False, debug=False,
                   num_devices=N_CORES)
    xT = nc.dram_tensor("xT", [H, NT], F32, kind="ExternalInput").ap()
    wg = nc.dram_tensor("wg", [H, E], F32, kind="ExternalInput").ap()
    bgr = nc.dram_tensor("bgr", [128, E], F32, kind="ExternalInput").ap()
    gate = nc.dram_tensor("gate", [NT, E], F32, kind="ExternalOutput").ap()

    with tile.TileContext(nc) as tc, ExitStack() as ctx:
        const_pool = ctx.enter_context(tc.tile_pool(name="const", bufs=1))
        wg_sb = const_pool.tile([128, E * KT], F32, tag="wg")
        nc.sync.dma_start(wg_sb[:].rearrange("p (k e) -> p k e", k=KT),
                          wg.rearrange("(k p) e -> p k e", p=128))
        bg_sb = const_pool.tile([128, E], F32, tag="bg")
        nc.sync.dma_start(bg_sb[:], bgr)

        with tc.tile_pool(name="gx", bufs=KT) as gx_pool, \
             tc.tile_pool(name="gsb", bufs=3) as gsb, \
             tc.tile_pool(name="gps", bufs=4, space="PSUM") as gps:
            xks = []
            for k in range(KT):
                xk = gx_pool.tile([128, NT], F32, tag="xg")
                nc.sync.dma_start(xk[:], xT[k * 128:(k + 1) * 128, :])
                xks.append(xk)
            for i in range(NT // 128):
                lg = gps.tile([128, E], F32, tag="lg")
                for k in range(KT):
                    nc.tensor.matmul(
                        lg[:], xks[k][:, i * 128:(i + 1) * 128],
                        wg_sb[:, k * E:(k + 1) * E],
                        start=(k == 0), stop=(k == KT - 1))
                L = gsb.tile([128, E], F32, tag="L")
                nc.vector.tensor_tensor(L[:], lg[:], bg_sb[:], OP.add)
                m1 = gsb.tile([128, 1], F32, tag="m1")
                nc.vector.tensor_reduce(m1[:], L[:], axis=mybir.AxisListType.X,
                                        op=OP.max)
                negm = gsb.tile([128, 1], F32, tag="negm")
                nc.vector.tensor_scalar(negm[:], m1[:], -1.0, None, OP.mult)
                ex = gsb.tile([128, E], F32, tag="ex")
                nc.scalar.activation(ex[:], L[:], AF.Exp, bias=negm[:],
                                     scale=1.0)
                s = gsb.tile([128, 1], F32, tag="s")
                nc.vector.tensor_reduce(s[:], ex[:], axis=mybir.AxisListType.X,
                                        op=OP.add)
                sinv = gsb.tile([128, 1], F32, tag="sinv")
                nc.vector.reciprocal(sinv[:], s[:])
                p = gsb.tile([128, E], F32, tag="p")
                nc.vector.tensor_scalar(p[:], ex[:], sinv[:], None, OP.mult)
                nc.sync.dma_start(gate[i * 128:(i + 1) * 128, :], p[:])

    nc.compile()
    nc.m = get_hw_module(nc.m)
    return nc


def _build_ffn():
    nc = bacc.Bacc("TRN2", target_bir_lowering=False, debug=False,
                   num_devices=N_CORES)
    xTe = nc.dram_tensor("xTe", [H, C], BF16, kind="ExternalInput").ap()
    w1e = nc.dram_tensor("w1e", [H, H], BF16, kind="ExternalInput").ap()
    w2e = nc.dram_tensor("w2e", [H, H], BF16, kind="ExternalInput").ap()
    wrow = nc.dram_tensor("wrow", [1, C], F32, kind="ExternalInput").ap()
    vec_in = {}
    for nm in ("b1", "g1", "be1", "b2", "g2", "be2"):
        vec_in[nm] = nc.dram_tensor(nm, [H], F32, kind="ExternalInput").ap()
    oute = nc.dram_tensor("oute", [H, C], F32, kind="ExternalOutput").ap()

    NCH = C // TS
    with tile.TileContext(nc) as tc, ExitStack() as ctx:
        const_pool = ctx.enter_context(tc.tile_pool(name="const", bufs=1))
        wt1 = const_pool.tile([128, KT * H], BF16, tag="wt1")
        wt2 = const_pool.tile([128, KT * H], BF16, tag="wt2")
        for k in range(KT):
            nc.sync.dma_start(wt1[:, k * H:(k + 1) * H],
                              w1e[k * 128:(k + 1) * 128, :])
        vecs = {}
        for nm in ("b1", "g1", "be1", "b2", "g2", "be2"):
            v = const_pool.tile([128, KT], F32, tag=f"v{nm}")
            nc.sync.dma_start(v[:], vec_in[nm].rearrange("(m p) -> p m", p=128))
            vecs[nm] = v
        ones_bf = const_pool.tile([128, 1], BF16, tag="ones_bf")
        nc.vector.memset(ones_bf[:], 1.0 / H)
        ones_orow = const_pool.tile([1, 128], BF16, tag="ones_orow")
        nc.vector.memset(ones_orow[:], 1.0)
        ones_orow32 = const_pool.tile([1, 128], F32, tag="ones_orow32")
        nc.vector.memset(ones_orow32[:], 1.0)
        eps_t = const_pool.tile([1, 1], F32, tag="eps")
        nc.vector.memset(eps_t[:], LN_EPS)

        with tc.tile_pool(name="xb", bufs=20) as xb_pool, \
             tc.tile_pool(name="hb", bufs=12) as hb_pool, \
             tc.tile_pool(name="sq", bufs=12) as sq_pool, \
             tc.tile_pool(name="tt", bufs=10) as t_pool, \
             tc.tile_pool(name="vf", bufs=10) as vf_pool, \
             tc.tile_pool(name="nt", bufs=6) as nt_pool, \
             tc.tile_pool(name="ab", bufs=4) as ab_pool, \
             tc.tile_pool(name="zz", bufs=8) as z_pool, \
             tc.tile_pool(name="row", bufs=8) as row_pool, \
             tc.tile_pool(name="wr", bufs=3) as wr_pool, \
             tc.tile_pool(name="mmp", bufs=5, space="PSUM") as mm_ps, \
             tc.tile_pool(name="mm2p", bufs=5, space="PSUM") as mm2_ps, \
             tc.tile_pool(name="stp", bufs=1, space="PSUM") as st_ps, \
             tc.tile_pool(name="rpp", bufs=2, space="PSUM") as rep_ps:

            def mm_phase(xin16, wt, pool, k_outer=False):
                pss = []
                if not k_outer:
                    for m in range(KT):
                        ps = pool.tile([128, TS], F32, tag="mm")
                        for k in range(KT):
                            nc.tensor.matmul(
                                ps[:],
                                wt[:, k * H + m * 128:k * H + (m + 1) * 128],
                                xin16[k][:], start=(k == 0),
                                stop=(k == KT - 1))
                        pss.append(ps)
                    return pss
                for g in range(2):
                    gps_tiles = []
                    for m in range(4):
                        ps = pool.tile([128, TS], F32, tag="mm")
                        gps_tiles.append(ps)
                    for k in range(KT):
                        for mi in range(4):
                            m = g * 4 + mi
                            nc.tensor.matmul(
                                gps_tiles[mi][:],
                                wt[:, k * H + m * 128:k * H + (m + 1) * 128],
                                xin16[k][:], start=(k == 0),
                                stop=(k == KT - 1))
                    pss.extend(gps_tiles)
                return pss

            def epilogue(pss, bias, gain, beta, store_bf16):
                hbs, sq_list = [], []
                for m in range(KT):
                    hb = hb_pool.tile([128, TS], BF16, tag="hb")
                    nc.scalar.activation(hb[:], pss[m][:], AF.Identity,
                                         bias=bias[:, m:m + 1], scale=1.0)
                    sq = sq_pool.tile([128, TS], BF16, tag="sq")
                    nc.gpsimd.tensor_tensor(sq[:], hb[:], hb[:], OP.mult)
                    hbs.append(hb)
                    sq_list.append(sq)
                s12 = st_ps.tile([33, TS], F32, tag="s1")
                s1 = s12[0:1, :]
                s2 = s12[32:33, :]
                for m in range(KT):
                    nc.tensor.matmul(s1, ones_bf[:], hbs[m][:],
                                     start=(m == 0), stop=(m == KT - 1))
                for m in range(KT):
                    nc.tensor.matmul(s2, ones_bf[:], sq_list[m][:],
                                     start=(m == 0), stop=(m == KT - 1))
                # mean path starts immediately after S1 stats (parallel
                # with the var->sqrt->recip chain)
                m16row = row_pool.tile([1, TS], BF16, tag="row16")
                nc.scalar.activation(m16row[:], s1, AF.Copy)
                m_ps = rep_ps.tile([128, TS], F32, tag="rep")
                nc.tensor.matmul(m_ps[:], ones_orow[:], m16row[:], start=True,
                                 stop=True)
                m16 = ab_pool.tile([128, TS], BF16, tag="ab")
                nc.scalar.activation(m16[:], m_ps[:], AF.Copy)
                s1r = row_pool.tile([1, TS], F32, tag="row")
                nc.vector.tensor_copy(s1r[:], s1)
                var = row_pool.tile([1, TS], F32, tag="row")
                nc.vector.scalar_tensor_tensor(var[:], s1r[:], -1.0, s1r[:],
                                               OP.mult, OP.mult)
                nc.vector.tensor_tensor(var[:], s2, var[:], OP.add)
                stdv = row_pool.tile([1, TS], F32, tag="row")
                nc.scalar.activation(stdv[:], var[:], AF.Sqrt, bias=eps_t[:],
                                     scale=1.0)
                arow = row_pool.tile([1, TS], BF16, tag="row16")
                with nc.allow_low_precision(reason="rstd row replicated as bf16"):
                    nc.vector.reciprocal(arow[:], stdv[:])
                a_ps = rep_ps.tile([128, TS], F32, tag="rep")
                nc.tensor.matmul(a_ps[:], ones_orow[:], arow[:], start=True,
                                 stop=True)
                a16 = ab_pool.tile([128, TS], BF16, tag="ab")
                nc.scalar.activation(a16[:], a_ps[:], AF.Copy)

                outs = []
                for m in range(KT):
                    nt = nt_pool.tile([128, TS], BF16, tag="n")
                    nc.vector.tensor_tensor(nt[:], hbs[m][:], m16[:],
                                            OP.subtract)
                    nc.vector.tensor_tensor(nt[:], nt[:], a16[:], OP.mult)
                    if store_bf16:
                        o = t_pool.tile([128, TS], BF16, tag="t")
                    else:
                        o = vf_pool.tile([128, TS], BF16, tag="v")
                    nc.scalar.activation(
                        o[:], nt[:], AF.Relu if store_bf16 else AF.Identity,
                        bias=beta[:, m:m + 1], scale=gain[:, m:m + 1])
                    outs.append(o)
                return outs

            def load_x(c):
                tsl = slice(c * TS, (c + 1) * TS)
                x16 = []
                for k in range(KT):
                    xb = xb_pool.tile([128, TS], BF16, tag="x16")
                    nc.sync.dma_start(xb[:], xTe[k * 128:(k + 1) * 128, tsl])
                    x16.append(xb)
                return x16

            def combine(c, x16, vfs):
                tsl = slice(c * TS, (c + 1) * TS)
                wre = wr_pool.tile([1, TS], F32, tag="wre")
                nc.sync.dma_start(wre[:], wrow[:, tsl])
                w_ps = rep_ps.tile([128, TS], F32, tag="rep")
                nc.tensor.matmul(w_ps[:], ones_orow32[:], wre[:], start=True,
                                 stop=True)
                for m in range(KT):
                    v = vfs[m]
                    nc.vector.tensor_tensor(v[:], v[:], x16[m][:], OP.add)
                    z = z_pool.tile([128, TS], F32, tag="z")
                    nc.vector.scalar_tensor_tensor(z[:], v[:], 0.0, w_ps[:],
                                                   OP.max, OP.mult)
                    nc.sync.dma_start(oute[m * 128:(m + 1) * 128, tsl], z[:])

            for c in range(NCH):
                x16 = load_x(c)
                ps1 = mm_phase(x16, wt1, mm_ps)
                if c == 0:
                    # deferred so chunk-0 x DMAs aren't queued behind 2MB of
                    # layer-2 weights nobody needs for ~30us
                    for k in range(KT):
                        nc.sync.dma_start(wt2[:, k * H:(k + 1) * H],
                                          w2e[k * 128:(k + 1) * 128, :])
                t16 = epilogue(ps1, vecs["b1"], vecs["g1"], vecs["be1"], True)
                ps2 = mm_phase(t16, wt2, mm_ps)
                vfs = epilogue(ps2, vecs["b2"], vecs["g2"], vecs["be2"], False)
                combine(c, x16, vfs)

    nc.compile()
    nc.m = get_hw_module(nc.m)
    return nc


def _ffn_host_exact(x_tok, W1e, b1e, g1e, be1e, W2e, b2e, g2e, be2e):
    """Exact f32 single-expert residual block for overflow tokens."""
    h = x_tok @ W1e + b1e
    mu = h.mean(-1, keepdims=True)
    va = ((h - mu) ** 2).mean(-1, keepdims=True)
    t = np.maximum((h - mu) / np.sqrt(va + LN_EPS) * g1e + be1e, 0)
    h2 = t @ W2e + b2e
    mu2 = h2.mean(-1, keepdims=True)
    va2 = ((h2 - mu2) ** 2).mean(-1, keepdims=True)
    v = (h2 - mu2) / np.sqrt(va2 + LN_EPS) * g2e + be2e
    return np.maximum(x_tok + v, 0)


def kernel(x, Wg, bg, W1, b1, g1, be1, W2, b2, g2, be2,
           _trace=False, _tmpdir=None):
    x = np.ascontiguousarray(np.asarray(x, np.float32))
    Wg = np.asarray(Wg, np.float32)
    bg = np.asarray(bg, np.float32)
    W1 = np.asarray(W1, np.float32)
    W2 = np.asarray(W2, np.float32)
    vec = {nm: np.ascontiguousarray(np.asarray(v, np.float32))
           for nm, v in (("b1", b1), ("g1", g1), ("be1", be1),
                         ("b2", b2), ("g2", g2), ("be2", be2))}

    # ---- phase A: gate on device ----
    if "gate" not in _CACHE:
        _CACHE["gate"] = _build_gate()
    gshared = {
        "wg": np.ascontiguousarray(Wg),
        "bgr": np.ascontiguousarray(np.broadcast_to(bg[None, :], (128, E))),
    }
    gmaps = []
    for cix in range(N_CORES):
        m = dict(gshared)
        m["xT"] = np.ascontiguousarray(x[cix * NT:(cix + 1) * NT].T)
        gmaps.append(m)
    gres = run_bass_kernel_spmd(_CACHE["gate"], gmaps, list(range(N_CORES)))
    gate = np.concatenate([gres.results[cix]["gate"]
                           for cix in range(N_CORES)], axis=0)

    # ---- host routing (selection identical to reference's top_k on probs) ----
    p = gate
    m1 = p.max(1, keepdims=True)
    pmask = np.where(p >= m1, -np.inf, p)
    m2 = pmask.max(1, keepdims=True).astype(np.float32)
    topmask = p >= m2                                   # [N, E] top-2 mask
    w = (p * topmask / (m1 + m2 + 1e-9)).astype(np.float32)

    W1b = W1.astype(ml_dtypes.bfloat16)
    W2b = W2.astype(ml_dtypes.bfloat16)
    xb16 = x.astype(ml_dtypes.bfloat16)

    idx_list, over_list = [], []
    fmaps = []
    for e in range(E):
        idx = np.nonzero(topmask[:, e])[0]
        over = idx[C:] if len(idx) > C else idx[:0]
        idx = idx[:C]
        idx_list.append(idx)
        over_list.append(over)
        xe = np.zeros((C, H), ml_dtypes.bfloat16)
        xe[:len(idx)] = xb16[idx]
        wr = np.zeros((1, C), np.float32)
        wr[0, :len(idx)] = w[idx, e]
        fmaps.append({
            "xTe": np.ascontiguousarray(xe.T),
            "w1e": np.ascontiguousarray(W1b[e]),
            "w2e": np.ascontiguousarray(W2b[e]),
            "wrow": wr,
            "b1": vec["b1"][e], "g1": vec["g1"][e], "be1": vec["be1"][e],
            "b2": vec["b2"][e], "g2": vec["g2"][e], "be2": vec["be2"][e],
        })

    # ---- phase B: expert-parallel FFN on device ----
    if "ffn" not in _CACHE:
        _CACHE["ffn"] = _build_ffn()
    fres = run_bass_kernel_spmd(_CACHE["ffn"], fmaps, list(range(N_CORES)),
                                trace=_trace, tmpdir=_tmpdir)
    _CACHE["last"] = fres

    moe = np.zeros((N, H), np.float32)
    for e in range(E):
        idx = idx_list[e]
        oe = fres.results[e]["oute"]                    # [H, C] f32
        moe[idx] += oe[:, :len(idx)].T
        if len(over_list[e]):
            ov = over_list[e]
            y = _ffn_host_exact(x[ov], W1[e], vec["b1"][e], vec["g1"][e],
                                vec["be1"][e], W2[e], vec["b2"][e],
                                vec["g2"][e], vec["be2"][e])
            moe[ov] += w[ov, e][:, None] * y
    return moe, gate


# revision 32
# speedup vs baseline: 1.0600x; 1.0600x over previous
"""MoE layer (N=16384, H=1024, E=8, top-2) on 8 TRN2 NeuronCores.

Two-phase sparse expert-parallel strategy:
  Phase A (gate): data-parallel over tokens (2048/core). fp32 gate matmul on
    PE + exact softmax -> gate_probs (bit-exact vs f32 reference on this HW).
  Host routing: top-2 selection from the returned probs (pure argsort-free
    masking, matches reference semantics), per-expert token gather, pad to
    capacity C, transpose to feature-major, cast to bf16.
  Phase B (FFN): expert-parallel - core e runs expert e's residual block over
    its C gathered tokens. Feature-major [H, tok] layout; LayerNorm over H via
    ones-matmul on PE; per-token stats replicated across partitions by PE
    outer products; combine weight applied on device; host scatter-adds the
    two contributions per token.
  Tokens beyond capacity (never expected for this distribution) are computed
  exactly on the host in f32.
"""

import sys

for _p in ("/opt/trn_rl_repo", "/root/.axon_site/_ro/trn_rl_repo"):
    if _p not in sys.path:
        sys.path.append(_p)

from contextlib import ExitStack

import ml_dtypes
import numpy as np

import concourse.bass as bass
import concourse.tile as tile
from concourse import bacc, mybir
from concourse.bass_interp import get_hw_module
from concourse.bass_utils import run_bass_kernel_spmd

F32 = mybir.dt.float32
BF16 = mybir.dt.bfloat16
AF = mybir.ActivationFunctionType
OP = mybir.AluOpType

N_CORES = 8
N, H, E = 16384, 1024, 8
NT = N // N_CORES          # tokens per core in gate phase (2048)
C = 4608                   # per-expert token capacity in FFN phase
TS = 512                   # token chunk (matmul free dim)
KT = H // 128              # 8 k/m tiles
LN_EPS = 1e-5

_CACHE = {}


def _build_gate():
    nc = bacc.Bacc("TRN2", target_bir_lowering=False, debug=False,
                   num_devices=N_CORES)
    xT = nc.dram_tensor("xT", [H, NT], F32, kind="ExternalInput").ap()
    wg = nc.dram_tensor("wg", [H, E], F32, kind="ExternalInput").ap()
    bgr = nc.dram_tensor("bgr", [128, E], F32, kind="ExternalInput").ap()
    gate = nc.dram_tensor("gate", [NT, E], F32, kind="ExternalOutput").ap()

    with tile.TileContext(nc) as tc, ExitStack() as ctx:
        const_pool = ctx.enter_context(tc.tile_pool(name="const", bufs=1))
        wg_sb = const_pool.tile([128, E * KT], F32, tag="wg")
        nc.sync.dma_start(wg_sb[:].rearrange("p (k e) -> p k e", k=KT),
                          wg.rearrange("(k p) e -> p k e", p=128))
        bg_sb = const_pool.tile([128, E], F32, tag="bg")
        nc.sync.dma_start(bg_sb[:], bgr)

        with tc.tile_pool(name="gx", bufs=KT) as gx_pool, \
             tc.tile_pool(name="gsb", bufs=3) as gsb, \
             tc.tile_pool(name="gps", bufs=4, space="PSUM") as gps:
            xks = []
            for k in range(KT):
                xk = gx_pool.tile([128, NT], F32, tag="xg")
                nc.sync.dma_start(xk[:], xT[k * 128:(k + 1) * 128, :])
                xks.append(xk)
            for i in range(NT // 128):
                lg = gps.tile([128, E], F32, tag="lg")
                for k in range(KT):
                    nc.tensor.matmul(
                        lg[:], xks[k][:, i * 128:(i + 1) * 128],
                        wg_sb[:, k * E:(k + 1) * E],
                        start=(k == 0), stop=(k == KT - 1))
                L = gsb.tile([128, E], F32, tag="L")
                nc.vector.tensor_tensor(L[:], lg[:], bg_sb[:], OP.add)
                m1 = gsb.tile([128, 1], F32, tag="m1")
                nc.vector.tensor_reduce(m1[:], L[:], axis=mybir.AxisListType.X,
                                        op=OP.max)
                negm = gsb.tile([128, 1], F32, tag="negm")
                nc.vector.tensor_scalar(negm[:], m1[:], -1.0, None, OP.mult)
                ex = gsb.tile([128, E], F32, tag="ex")
                nc.scalar.activation(ex[:], L[:], AF.Exp, bias=negm[:],
                                     scale=1.0)
                s = gsb.tile([128, 1], F32, tag="s")
                nc.vector.tensor_reduce(s[:], ex[:], axis=mybir.AxisListType.X,
                                        op=OP.add)
                sinv = gsb.tile([128, 1], F32, tag="sinv")
                nc.vector.reciprocal(sinv[:], s[:])
                p = gsb.tile([128, E], F32, tag="p")
                nc.vector.tensor_scalar(p[:], ex[:], sinv[:], None, OP.mult)
                nc.sync.dma_start(gate[i * 128:(i + 1) * 128, :], p[:])

    nc.compile()
    nc.m = get_hw_module(nc.m)
    return nc


def _build_ffn():
    nc = bacc.Bacc("TRN2", target_bir_lowering=False, debug=False,
                   num_devices=N_CORES)
    xTe = nc.dram_tensor("xTe", [H, C], BF16, kind="ExternalInput").ap()
    w1e = nc.dram_tensor("w1e", [H, H], BF16, kind="ExternalInput").ap()
    w2e = nc.dram_tensor("w2e", [H, H], BF16, kind="ExternalInput").ap()
    wrow = nc.dram_tensor("wrow", [1, C], F32, kind="ExternalInput").ap()
    vec_in = {}
    for nm in ("b1", "g1", "be1", "b2", "g2", "be2"):
        vec_in[nm] = nc.dram_tensor(nm, [H], F32, kind="ExternalInput").ap()
    oute = nc.dram_tensor("oute", [H, C], F32, kind="ExternalOutput").ap()

    NCH = C // TS
    with tile.TileContext(nc) as tc, ExitStack() as ctx:
        const_pool = ctx.enter_context(tc.tile_pool(name="const", bufs=1))
        wt1 = const_pool.tile([128, KT * H], BF16, tag="wt1")
        wt2 = const_pool.tile([128, KT * H], BF16, tag="wt2")
        for k in range(KT):
            nc.sync.dma_start(wt1[:, k * H:(k + 1) * H],
                              w1e[k * 128:(k + 1) * 128, :])
        vecs = {}
        for nm in ("b1", "g1", "be1", "b2", "g2", "be2"):
            v = const_pool.tile([128, KT], F32, tag=f"v{nm}")
            nc.sync.dma_start(v[:], vec_in[nm].rearrange("(m p) -> p m", p=128))
            vecs[nm] = v
        ones_bf = const_pool.tile([128, 1], BF16, tag="ones_bf")
        nc.vector.memset(ones_bf[:], 1.0 / H)
        ones_orow = const_pool.tile([1, 128], BF16, tag="ones_orow")
        nc.vector.memset(ones_orow[:], 1.0)
        ones_orow32 = const_pool.tile([1, 128], F32, tag="ones_orow32")
        nc.vector.memset(ones_orow32[:], 1.0)
        eps_t = const_pool.tile([1, 1], F32, tag="eps")
        nc.vector.memset(eps_t[:], LN_EPS)

        with tc.tile_pool(name="xb", bufs=20) as xb_pool, \
             tc.tile_pool(name="hb", bufs=12) as hb_pool, \
             tc.tile_pool(name="sq", bufs=12) as sq_pool, \
             tc.tile_pool(name="tt", bufs=10) as t_pool, \
             tc.tile_pool(name="vf", bufs=10) as vf_pool, \
             tc.tile_pool(name="nt", bufs=6) as nt_pool, \
             tc.tile_pool(name="ab", bufs=4) as ab_pool, \
             tc.tile_pool(name="zz", bufs=8) as z_pool, \
             tc.tile_pool(name="row", bufs=8) as row_pool, \
             tc.tile_pool(name="wr", bufs=3) as wr_pool, \
             tc.tile_pool(name="mmp", bufs=5, space="PSUM") as mm_ps, \
             tc.tile_pool(name="mm2p", bufs=5, space="PSUM") as mm2_ps, \
             tc.tile_pool(name="stp", bufs=1, space="PSUM") as st_ps, \
             tc.tile_pool(name="rpp", bufs=2, space="PSUM") as rep_ps:

            def mm_phase(xin16, wt, pool, k_outer=False):
                pss = []
                if not k_outer:
                    for m in range(KT):
                        ps = pool.tile([128, TS], F32, tag="mm")
                        for k in range(KT):
                            nc.tensor.matmul(
                                ps[:],
                                wt[:, k * H + m * 128:k * H + (m + 1) * 128],
                                xin16[k][:], start=(k == 0),
                                stop=(k == KT - 1))
                        pss.append(ps)
                    return pss
                for g in range(2):
                    gps_tiles = []
                    for m in range(4):
                        ps = pool.tile([128, TS], F32, tag="mm")
                        gps_tiles.append(ps)
                    for k in range(KT):
                        for mi in range(4):
                            m = g * 4 + mi
                            nc.tensor.matmul(
                                gps_tiles[mi][:],
                                wt[:, k * H + m * 128:k * H + (m + 1) * 128],
                                xin16[k][:], start=(k == 0),
                                stop=(k == KT - 1))
                    pss.extend(gps_tiles)
                return pss

            def epilogue(pss, bias, gain, beta, store_bf16):
                hbs, sq_list = [], []
                for m in range(KT):
                    hb = hb_pool.tile([128, TS], BF16, tag="hb")
                    nc.scalar.activation(hb[:], pss[m][:], AF.Identity,
                                         bias=bias[:, m:m + 1], scale=1.0)
                    sq = sq_pool.tile([128, TS], BF16, tag="sq")
                    nc.gpsimd.tensor_tensor(sq[:], hb[:], hb[:], OP.mult)
                    hbs.append(hb)
                    sq_list.append(sq)
                s12 = st_ps.tile([33, TS], F32, tag="s1")
                s1 = s12[0:1, :]
                s2 = s12[32:33, :]
                for m in range(KT):
                    nc.tensor.matmul(s1, ones_bf[:], hbs[m][:],
                                     start=(m == 0), stop=(m == KT - 1))
                for m in range(KT):
                    nc.tensor.matmul(s2, ones_bf[:], sq_list[m][:],
                                     start=(m == 0), stop=(m == KT - 1))
                # mean path starts immediately after S1 stats (parallel
                # with the var->sqrt->recip chain)
                m16row = row_pool.tile([1, TS], BF16, tag="row16")
                nc.scalar.activation(m16row[:], s1, AF.Copy)
                m_ps = rep_ps.tile([128, TS], F32, tag="rep")
                nc.tensor.matmul(m_ps[:], ones_orow[:], m16row[:], start=True,
                                 stop=True)
                m16 = ab_pool.tile([128, TS], BF16, tag="ab")
                nc.scalar.activation(m16[:], m_ps[:], AF.Copy)
                s1sq = row_pool.tile([1, TS], F32, tag="row")
                nc.scalar.activation(s1sq[:], s1, AF.Square)
                var = row_pool.tile([1, TS], F32, tag="row")
                nc.vector.tensor_tensor(var[:], s2, s1sq[:], OP.subtract)
                stdv = row_pool.tile([1, TS], F32, tag="row")
                nc.scalar.activation(stdv[:], var[:], AF.Sqrt, bias=eps_t[:],
                                     scale=1.0)
                arow = row_pool.tile([1, TS], BF16, tag="row16")
                with nc.allow_low_precision(reason="rstd row replicated as bf16"):
                    nc.vector.reciprocal(arow[:], stdv[:])
                a_ps = rep_ps.tile([128, TS], F32, tag="rep")
                nc.tensor.matmul(a_ps[:], ones_orow[:], arow[:], start=True,
                                 stop=True)
                a16 = ab_pool.tile([128, TS], BF16, tag="ab")
                nc.scalar.activation(a16[:], a_ps[:], AF.Copy)

                outs = []
                for m in range(KT):
                    nt = nt_pool.tile([128, TS], BF16, tag="n")
                    nc.vector.tensor_tensor(nt[:], hbs[m][:], m16[:],
                                            OP.subtract)
                    nc.vector.tensor_tensor(nt[:], nt[:], a16[:], OP.mult)
                    if store_bf16:
                        o = t_pool.tile([128, TS], BF16, tag="t")
                    else:
                        o = vf_pool.tile([128, TS], BF16, tag="v")
                    nc.scalar.activation(
                        o[:], nt[:], AF.Relu if store_bf16 else AF.Identity,
                        bias=beta[:, m:m + 1], scale=gain[:, m:m + 1])
                    outs.append(o)
                return outs

            def load_x(c):
                tsl = slice(c * TS, (c + 1) * TS)
                x16 = []
                for k in range(KT):
                    xb = xb_pool.tile([128, TS], BF16, tag="x16")
                    nc.sync.dma_start(xb[:], xTe[k * 128:(k + 1) * 128, tsl])
                    x16.append(xb)
                return x16

            def combine(c, x16, vfs):
                tsl = slice(c * TS, (c + 1) * TS)
                wre = wr_pool.tile([1, TS], F32, tag="wre")
                nc.sync.dma_start(wre[:], wrow[:, tsl])
                w_ps = rep_ps.tile([128, TS], F32, tag="rep")
                nc.tensor.matmul(w_ps[:], ones_orow32[:], wre[:], start=True,
                                 stop=True)
                for m in range(KT):
                    v = vfs[m]
                    nc.vector.tensor_tensor(v[:], v[:], x16[m][:], OP.add)
                    z = z_pool.tile([128, TS], F32, tag="z")
                    nc.vector.scalar_tensor_tensor(z[:], v[:], 0.0, w_ps[:],
                                                   OP.max, OP.mult)
                    nc.sync.dma_start(oute[m * 128:(m + 1) * 128, tsl], z[:])

            for c in range(NCH):
                x16 = load_x(c)
                ps1 = mm_phase(x16, wt1, mm_ps)
                if c == 0:
                    # deferred so chunk-0 x DMAs aren't queued behind 2MB of
                    # layer-2 weights nobody needs for ~30us
                    for k in range(KT):
                        nc.sync.dma_start(wt2[:, k * H:(k + 1) * H],
                                          w2e[k * 128:(k + 1) * 128, :])
                t16 = epilogue(ps1, vecs["b1"], vecs["g1"], vecs["be1"], True)
                ps2 = mm_phase(t16, wt2, mm_ps)
                vfs = epilogue(ps2, vecs["b2"], vecs["g2"], vecs["be2"], False)
                combine(c, x16, vfs)

    nc.compile()
    nc.m = get_hw_module(nc.m)
    return nc


def _ffn_host_exact(x_tok, W1e, b1e, g1e, be1e, W2e, b2e, g2e, be2e):
    """Exact f32 single-expert residual block for overflow tokens."""
    h = x_tok @ W1e + b1e
    mu = h.mean(-1, keepdims=True)
    va = ((h - mu) ** 2).mean(-1, keepdims=True)
    t = np.maximum((h - mu) / np.sqrt(va + LN_EPS) * g1e + be1e, 0)
    h2 = t @ W2e + b2e
    mu2 = h2.mean(-1, keepdims=True)
    va2 = ((h2 - mu2) ** 2).mean(-1, keepdims=True)
    v = (h2 - mu2) / np.sqrt(va2 + LN_EPS) * g2e + be2e
    return np.maximum(x_tok + v, 0)


def kernel(x, Wg, bg, W1, b1, g1, be1, W2, b2, g2, be2,
           _trace=False, _tmpdir=None):
    x = np.ascontiguousarray(np.asarray(x, np.float32))
    Wg = np.asarray(Wg, np.float32)
    bg = np.asarray(bg, np.float32)
    W1 = np.asarray(W1, np.float32)
    W2 = np.asarray(W2, np.float32)
    vec = {nm: np.ascontiguousarray(np.asarray(v, np.float32))
           for nm, v in (("b1", b1), ("g1", g1), ("be1", be1),
                         ("b2", b2), ("g2", g2), ("be2", be2))}

    # ---- phase A: gate on device ----
    if "gate" not in _CACHE:
        _CACHE["gate"] = _build_gate()
    gshared = {
        "wg": np.ascontiguousarray(Wg),
        "bgr": np.ascontiguousarray(np.broadcast_to(bg[None, :], (128, E))),
    }
    gmaps = []
    for cix in range(N_CORES):
        m = dict(gshared)
        m["xT"] = np.ascontiguousarray(x[cix * NT:(cix + 1) * NT].T)
        gmaps.append(m)
    gres = run_bass_kernel_spmd(_CACHE["gate"], gmaps, list(range(N_CORES)))
    gate = np.concatenate([gres.results[cix]["gate"]
                           for cix in range(N_CORES)], axis=0)

    # ---- host routing (selection identical to reference's top_k on probs) ----
    p = gate
    m1 = p.max(1, keepdims=True)
    pmask = np.where(p >= m1, -np.inf, p)
    m2 = pmask.max(1, keepdims=True).astype(np.float32)
    topmask = p >= m2                                   # [N, E] top-2 mask
    w = (p * topmask / (m1 + m2 + 1e-9)).astype(np.float32)

    W1b = W1.astype(ml_dtypes.bfloat16)
    W2b = W2.astype(ml_dtypes.bfloat16)
    xb16 = x.astype(ml_dtypes.bfloat16)

    idx_list, over_list = [], []
    fmaps = []
    for e in range(E):
        idx = np.nonzero(topmask[:, e])[0]
        over = idx[C:] if len(idx) > C else idx[:0]
        idx = idx[:C]
        idx_list.append(idx)
        over_list.append(over)
        xe = np.zeros((C, H), ml_dtypes.bfloat16)
        xe[:len(idx)] = xb16[idx]
        wr = np.zeros((1, C), np.float32)
        wr[0, :len(idx)] = w[idx, e]
        fmaps.append({
            "xTe": np.ascontiguousarray(xe.T),
            "w1e": np.ascontiguousarray(W1b[e]),
            "w2e": np.ascontiguousarray(W2b[e]),
            "wrow": wr,
            "b1": vec["b1"][e], "g1": vec["g1"][e], "be1": vec["be1"][e],
            "b2": vec["b2"][e], "g2": vec["g2"][e], "be2": vec["be2"][e],
        })

    # ---- phase B: expert-parallel FFN on device ----
    if "ffn" not in _CACHE:
        _CACHE["ffn"] = _build_ffn()
    fres = run_bass_kernel_spmd(_CACHE["ffn"], fmaps, list(range(N_CORES)),
                                trace=_trace, tmpdir=_tmpdir)
    _CACHE["last"] = fres

    moe = np.zeros((N, H), np.float32)
    for e in range(E):
        idx = idx_list[e]
        oe = fres.results[e]["oute"]                    # [H, C] f32
        moe[idx] += oe[:, :len(idx)].T
        if len(over_list[e]):
            ov = over_list[e]
            y = _ffn_host_exact(x[ov], W1[e], vec["b1"][e], vec["g1"][e],
                                vec["be1"][e], W2[e], vec["b2"][e],
                                vec["g2"][e], vec["be2"][e])
            moe[ov] += w[ov, e][:, None] * y
    return moe, gate


# revision 33
# speedup vs baseline: 1.0635x; 1.0034x over previous
"""MoE layer (N=16384, H=1024, E=8, top-2) on 8 TRN2 NeuronCores.

Two-phase sparse expert-parallel strategy:
  Phase A (gate): data-parallel over tokens (2048/core). fp32 gate matmul on
    PE + exact softmax -> gate_probs (bit-exact vs f32 reference on this HW).
  Host routing: top-2 selection from the returned probs (pure argsort-free
    masking, matches reference semantics), per-expert token gather, pad to
    capacity C, transpose to feature-major, cast to bf16.
  Phase B (FFN): expert-parallel - core e runs expert e's residual block over
    its C gathered tokens. Feature-major [H, tok] layout; LayerNorm over H via
    ones-matmul on PE; per-token stats replicated across partitions by PE
    outer products; combine weight applied on device; host scatter-adds the
    two contributions per token.
  Tokens beyond capacity (never expected for this distribution) are computed
  exactly on the host in f32.
"""

import sys

for _p in ("/opt/trn_rl_repo", "/root/.axon_site/_ro/trn_rl_repo"):
    if _p not in sys.path:
        sys.path.append(_p)

from contextlib import ExitStack

import ml_dtypes
import numpy as np

import concourse.bass as bass
import concourse.tile as tile
from concourse import bacc, mybir
from concourse.bass_interp import get_hw_module
from concourse.bass_utils import run_bass_kernel_spmd

F32 = mybir.dt.float32
BF16 = mybir.dt.bfloat16
AF = mybir.ActivationFunctionType
OP = mybir.AluOpType

N_CORES = 8
N, H, E = 16384, 1024, 8
NT = N // N_CORES          # tokens per core in gate phase (2048)
C = 4608                   # per-expert token capacity in FFN phase
TS = 512                   # token chunk (matmul free dim)
KT = H // 128              # 8 k/m tiles
LN_EPS = 1e-5

_CACHE = {}


def _build_gate():
    nc = bacc.Bacc("TRN2", target_bir_lowering=False, debug=False,
                   num_devices=N_CORES)
    xT = nc.dram_tensor("xT", [H, NT], F32, kind="ExternalInput").ap()
    wg = nc.dram_tensor("wg", [H, E], F32, kind="ExternalInput").ap()
    bgr = nc.dram_tensor("bgr", [128, E], F32, kind="ExternalInput").ap()
    gate = nc.dram_tensor("gate", [NT, E], F32, kind="ExternalOutput").ap()

    with tile.TileContext(nc) as tc, ExitStack() as ctx:
        const_pool = ctx.enter_context(tc.tile_pool(name="const", bufs=1))
        wg_sb = const_pool.tile([128, E * KT], F32, tag="wg")
        nc.sync.dma_start(wg_sb[:].rearrange("p (k e) -> p k e", k=KT),
                          wg.rearrange("(k p) e -> p k e", p=128))
        bg_sb = const_pool.tile([128, E], F32, tag="bg")
        nc.sync.dma_start(bg_sb[:], bgr)

        with tc.tile_pool(name="gx", bufs=KT) as gx_pool, \
             tc.tile_pool(name="gsb", bufs=3) as gsb, \
             tc.tile_pool(name="gps", bufs=4, space="PSUM") as gps:
            xks = []
            for k in range(KT):
                xk = gx_pool.tile([128, NT], F32, tag="xg")
                nc.sync.dma_start(xk[:], xT[k * 128:(k + 1) * 128, :])
                xks.append(xk)
            for i in range(NT // 128):
                lg = gps.tile([128, E], F32, tag="lg")
                for k in range(KT):
                    nc.tensor.matmul(
                        lg[:], xks[k][:, i * 128:(i + 1) * 128],
                        wg_sb[:, k * E:(k + 1) * E],
                        start=(k == 0), stop=(k == KT - 1))
                L = gsb.tile([128, E], F32, tag="L")
                nc.vector.tensor_tensor(L[:], lg[:], bg_sb[:], OP.add)
                m1 = gsb.tile([128, 1], F32, tag="m1")
                nc.vector.tensor_reduce(m1[:], L[:], axis=mybir.AxisListType.X,
                                        op=OP.max)
                negm = gsb.tile([128, 1], F32, tag="negm")
                nc.vector.tensor_scalar(negm[:], m1[:], -1.0, None, OP.mult)
                ex = gsb.tile([128, E], F32, tag="ex")
                nc.scalar.activation(ex[:], L[:], AF.Exp, bias=negm[:],
                                     scale=1.0)
                s = gsb.tile([128, 1], F32, tag="s")
                nc.vector.tensor_reduce(s[:], ex[:], axis=mybir.AxisListType.X,
                                        op=OP.add)
                sinv = gsb.tile([128, 1], F32, tag="sinv")
                nc.vector.reciprocal(sinv[:], s[:])
                p = gsb.tile([128, E], F32, tag="p")
                nc.vector.tensor_scalar(p[:], ex[:], sinv[:], None, OP.mult)
                nc.sync.dma_start(gate[i * 128:(i + 1) * 128, :], p[:])

    nc.compile()
    nc.m = get_hw_module(nc.m)
    return nc


def _build_ffn():
    nc = bacc.Bacc("TRN2", target_bir_lowering=False, debug=False,
                   num_devices=N_CORES)
    xTe = nc.dram_tensor("xTe", [H, C], BF16, kind="ExternalInput").ap()
    w1e = nc.dram_tensor("w1e", [H, H], BF16, kind="ExternalInput").ap()
    w2e = nc.dram_tensor("w2e", [H, H], BF16, kind="ExternalInput").ap()
    wrow = nc.dram_tensor("wrow", [1, C], F32, kind="ExternalInput").ap()
    vec_in = {}
    for nm in ("b1", "g1", "be1", "b2", "g2", "be2"):
        vec_in[nm] = nc.dram_tensor(nm, [H], F32, kind="ExternalInput").ap()
    oute = nc.dram_tensor("oute", [H, C], F32, kind="ExternalOutput").ap()

    NCH = C // TS
    with tile.TileContext(nc) as tc, ExitStack() as ctx:
        const_pool = ctx.enter_context(tc.tile_pool(name="const", bufs=1))
        wt1 = const_pool.tile([128, KT * H], BF16, tag="wt1")
        wt2 = const_pool.tile([128, KT * H], BF16, tag="wt2")
        for k in range(KT):
            nc.sync.dma_start(wt1[:, k * H:(k + 1) * H],
                              w1e[k * 128:(k + 1) * 128, :])
        vecs = {}
        for nm in ("b1", "g1", "be1", "b2", "g2", "be2"):
            v = const_pool.tile([128, KT], F32, tag=f"v{nm}")
            nc.sync.dma_start(v[:], vec_in[nm].rearrange("(m p) -> p m", p=128))
            vecs[nm] = v
        ones_bf = const_pool.tile([128, 1], BF16, tag="ones_bf")
        nc.vector.memset(ones_bf[:], 1.0 / H)
        ones_orow = const_pool.tile([1, 128], BF16, tag="ones_orow")
        nc.vector.memset(ones_orow[:], 1.0)
        ones_orow32 = const_pool.tile([1, 128], F32, tag="ones_orow32")
        nc.vector.memset(ones_orow32[:], 1.0)
        eps_t = const_pool.tile([1, 1], F32, tag="eps")
        nc.vector.memset(eps_t[:], LN_EPS)

        with tc.tile_pool(name="xb", bufs=20) as xb_pool, \
             tc.tile_pool(name="hb", bufs=12) as hb_pool, \
             tc.tile_pool(name="sq", bufs=12) as sq_pool, \
             tc.tile_pool(name="tt", bufs=10) as t_pool, \
             tc.tile_pool(name="vf", bufs=10) as vf_pool, \
             tc.tile_pool(name="nt", bufs=6) as nt_pool, \
             tc.tile_pool(name="ab", bufs=4) as ab_pool, \
             tc.tile_pool(name="zz", bufs=8) as z_pool, \
             tc.tile_pool(name="row", bufs=8) as row_pool, \
             tc.tile_pool(name="wr", bufs=3) as wr_pool, \
             tc.tile_pool(name="mmp", bufs=5, space="PSUM") as mm_ps, \
             tc.tile_pool(name="mm2p", bufs=5, space="PSUM") as mm2_ps, \
             tc.tile_pool(name="stp", bufs=1, space="PSUM") as st_ps, \
             tc.tile_pool(name="rpp", bufs=2, space="PSUM") as rep_ps:

            def mm_phase(xin16, wt, pool, k_outer=False):
                pss = []
                if not k_outer:
                    for m in range(KT):
                        ps = pool.tile([128, TS], F32, tag="mm")
                        for k in range(KT):
                            nc.tensor.matmul(
                                ps[:],
                                wt[:, k * H + m * 128:k * H + (m + 1) * 128],
                                xin16[k][:], start=(k == 0),
                                stop=(k == KT - 1))
                        pss.append(ps)
                    return pss
                for g in range(2):
                    gps_tiles = []
                    for m in range(4):
                        ps = pool.tile([128, TS], F32, tag="mm")
                        gps_tiles.append(ps)
                    for k in range(KT):
                        for mi in range(4):
                            m = g * 4 + mi
                            nc.tensor.matmul(
                                gps_tiles[mi][:],
                                wt[:, k * H + m * 128:k * H + (m + 1) * 128],
                                xin16[k][:], start=(k == 0),
                                stop=(k == KT - 1))
                    pss.extend(gps_tiles)
                return pss

            def epilogue(pss, bias, gain, beta, store_bf16):
                hbs, sq_list = [], []
                for m in range(KT):
                    hb = hb_pool.tile([128, TS], BF16, tag="hb")
                    nc.scalar.activation(hb[:], pss[m][:], AF.Identity,
                                         bias=bias[:, m:m + 1], scale=1.0)
                    sq = sq_pool.tile([128, TS], BF16, tag="sq")
                    eng = nc.gpsimd if m % 2 == 0 else nc.vector
                    eng.tensor_tensor(sq[:], hb[:], hb[:], OP.mult)
                    hbs.append(hb)
                    sq_list.append(sq)
                s12 = st_ps.tile([33, TS], F32, tag="s1")
                s1 = s12[0:1, :]
                s2 = s12[32:33, :]
                for m in range(KT):
                    nc.tensor.matmul(s1, ones_bf[:], hbs[m][:],
                                     start=(m == 0), stop=(m == KT - 1))
                for m in range(KT):
                    nc.tensor.matmul(s2, ones_bf[:], sq_list[m][:],
                                     start=(m == 0), stop=(m == KT - 1))
                # mean path starts immediately after S1 stats (parallel
                # with the var->sqrt->recip chain)
                m16row = row_pool.tile([1, TS], BF16, tag="row16")
                nc.scalar.activation(m16row[:], s1, AF.Copy)
                m_ps = rep_ps.tile([128, TS], F32, tag="rep")
                nc.tensor.matmul(m_ps[:], ones_orow[:], m16row[:], start=True,
                                 stop=True)
                m16 = ab_pool.tile([128, TS], BF16, tag="ab")
                nc.scalar.activation(m16[:], m_ps[:], AF.Copy)
                s1sq = row_pool.tile([1, TS], F32, tag="row")
                nc.scalar.activation(s1sq[:], s1, AF.Square)
                var = row_pool.tile([1, TS], F32, tag="row")
                nc.vector.tensor_tensor(var[:], s2, s1sq[:], OP.subtract)
                stdv = row_pool.tile([1, TS], F32, tag="row")
                nc.scalar.activation(stdv[:], var[:], AF.Sqrt, bias=eps_t[:],
                                     scale=1.0)
                arow = row_pool.tile([1, TS], BF16, tag="row16")
                with nc.allow_low_precision(reason="rstd row replicated as bf16"):
                    nc.vector.reciprocal(arow[:], stdv[:])
                a_ps = rep_ps.tile([128, TS], F32, tag="rep")
                nc.tensor.matmul(a_ps[:], ones_orow[:], arow[:], start=True,
                                 stop=True)
                a16 = ab_pool.tile([128, TS], BF16, tag="ab")
                nc.scalar.activation(a16[:], a_ps[:], AF.Copy)

                outs = []
                for m in range(KT):
                    nt = nt_pool.tile([128, TS], BF16, tag="n")
                    nc.vector.tensor_tensor(nt[:], hbs[m][:], m16[:],
                                            OP.subtract)
                    nc.vector.tensor_tensor(nt[:], nt[:], a16[:], OP.mult)
                    if store_bf16:
                        o = t_pool.tile([128, TS], BF16, tag="t")
                    else:
                        o = vf_pool.tile([128, TS], BF16, tag="v")
                    nc.scalar.activation(
                        o[:], nt[:], AF.Relu if store_bf16 else AF.Identity,
                        bias=beta[:, m:m + 1], scale=gain[:, m:m + 1])
                    outs.append(o)
                return outs

            def load_x(c):
                tsl = slice(c * TS, (c + 1) * TS)
                x16 = []
                for k in range(KT):
                    xb = xb_pool.tile([128, TS], BF16, tag="x16")
                    nc.sync.dma_start(xb[:], xTe[k * 128:(k + 1) * 128, tsl])
                    x16.append(xb)
                return x16

            def combine(c, x16, vfs):
                tsl = slice(c * TS, (c + 1) * TS)
                wre = wr_pool.tile([1, TS], F32, tag="wre")
                nc.sync.dma_start(wre[:], wrow[:, tsl])
                w_ps = rep_ps.tile([128, TS], F32, tag="rep")
                nc.tensor.matmul(w_ps[:], ones_orow32[:], wre[:], start=True,
                                 stop=True)
                for m in range(KT):
                    v = vfs[m]
                    nc.vector.tensor_tensor(v[:], v[:], x16[m][:], OP.add)
                    z = z_pool.tile([128, TS], F32, tag="z")
                    nc.vector.scalar_tensor_tensor(z[:], v[:], 0.0, w_ps[:],
                                                   OP.max, OP.mult)
                    nc.sync.dma_start(oute[m * 128:(m + 1) * 128, tsl], z[:])

            for c in range(NCH):
                x16 = load_x(c)
                ps1 = mm_phase(x16, wt1, mm_ps)
                if c == 0:
                    # deferred so chunk-0 x DMAs aren't queued behind 2MB of
                    # layer-2 weights nobody needs for ~30us
                    for k in range(KT):
                        nc.sync.dma_start(wt2[:, k * H:(k + 1) * H],
                                          w2e[k * 128:(k + 1) * 128, :])
                t16 = epilogue(ps1, vecs["b1"], vecs["g1"], vecs["be1"], True)
                ps2 = mm_phase(t16, wt2, mm_ps)
                vfs = epilogue(ps2, vecs["b2"], vecs["g2"], vecs["be2"], False)
                combine(c, x16, vfs)

    nc.compile()
    nc.m = get_hw_module(nc.m)
    return nc


def _ffn_host_exact(x_tok, W1e, b1e, g1e, be1e, W2e, b2e, g2e, be2e):
    """Exact f32 single-expert residual block for overflow tokens."""
    h = x_tok @ W1e + b1e
    mu = h.mean(-1, keepdims=True)
    va = ((h - mu) ** 2).mean(-1, keepdims=True)
    t = np.maximum((h - mu) / np.sqrt(va + LN_EPS) * g1e + be1e, 0)
    h2 = t @ W2e + b2e
    mu2 = h2.mean(-1, keepdims=True)
    va2 = ((h2 - mu2) ** 2).mean(-1, keepdims=True)
    v = (h2 - mu2) / np.sqrt(va2 + LN_EPS) * g2e + be2e
    return np.maximum(x_tok + v, 0)


def kernel(x, Wg, bg, W1, b1, g1, be1, W2, b2, g2, be2,
           _trace=False, _tmpdir=None):
    x = np.ascontiguousarray(np.asarray(x, np.float32))
    Wg = np.asarray(Wg, np.float32)
    bg = np.asarray(bg, np.float32)
    W1 = np.asarray(W1, np.float32)
    W2 = np.asarray(W2, np.float32)
    vec = {nm: np.ascontiguousarray(np.asarray(v, np.float32))
           for nm, v in (("b1", b1), ("g1", g1), ("be1", be1),
                         ("b2", b2), ("g2", g2), ("be2", be2))}

    # ---- phase A: gate on device ----
    if "gate" not in _CACHE:
        _CACHE["gate"] = _build_gate()
    gshared = {
        "wg": np.ascontiguousarray(Wg),
        "bgr": np.ascontiguousarray(np.broadcast_to(bg[None, :], (128, E))),
    }
    gmaps = []
    for cix in range(N_CORES):
        m = dict(gshared)
        m["xT"] = np.ascontiguousarray(x[cix * NT:(cix + 1) * NT].T)
        gmaps.append(m)
    gres = run_bass_kernel_spmd(_CACHE["gate"], gmaps, list(range(N_CORES)))
    gate = np.concatenate([gres.results[cix]["gate"]
                           for cix in range(N_CORES)], axis=0)

    # ---- host routing (selection identical to reference's top_k on probs) ----
    p = gate
    m1 = p.max(1, keepdims=True)
    pmask = np.where(p >= m1, -np.inf, p)
    m2 = pmask.max(1, keepdims=True).astype(np.float32)
    topmask = p >= m2                                   # [N, E] top-2 mask
    w = (p * topmask / (m1 + m2 + 1e-9)).astype(np.float32)

    W1b = W1.astype(ml_dtypes.bfloat16)
    W2b = W2.astype(ml_dtypes.bfloat16)
    xb16 = x.astype(ml_dtypes.bfloat16)

    idx_list, over_list = [], []
    fmaps = []
    for e in range(E):
        idx = np.nonzero(topmask[:, e])[0]
        over = idx[C:] if len(idx) > C else idx[:0]
        idx = idx[:C]
        idx_list.append(idx)
        over_list.append(over)
        xe = np.zeros((C, H), ml_dtypes.bfloat16)
        xe[:len(idx)] = xb16[idx]
        wr = np.zeros((1, C), np.float32)
        wr[0, :len(idx)] = w[idx, e]
        fmaps.append({
            "xTe": np.ascontiguousarray(xe.T),
            "w1e": np.ascontiguousarray(W1b[e]),
            "w2e": np.ascontiguousarray(W2b[e]),
            "wrow": wr,
            "b1": vec["b1"][e], "g1": vec["g1"][e], "be1": vec["be1"][e],
            "b2": vec["b2"][e], "g2": vec["g2"][e], "be2": vec["be2"][e],
        })

    # ---- phase B: expert-parallel FFN on device ----
    if "ffn" not in _CACHE:
        _CACHE["ffn"] = _build_ffn()
    fres = run_bass_kernel_spmd(_CACHE["ffn"], fmaps, list(range(N_CORES)),
                                trace=_trace, tmpdir=_tmpdir)
    _CACHE["last"] = fres

    moe = np.zeros((N, H), np.float32)
    for e in range(E):
        idx = idx_list[e]
        oe = fres.results[e]["oute"]                    # [H, C] f32
        moe[idx] += oe[:, :len(idx)].T
        if len(over_list[e]):
            ov = over_list[e]
            y = _ffn_host_exact(x[ov], W1[e], vec["b1"][e], vec["g1"][e],
                                vec["be1"][e], W2[e], vec["b2"][e],
                                vec["g2"][e], vec["be2"][e])
            moe[ov] += w[ov, e][:, None] * y
    return moe, gate


# revision 39
# speedup vs baseline: 1.0859x; 1.0210x over previous
"""MoE layer (N=16384, H=1024, E=8, top-2) on 8 TRN2 NeuronCores.

Two-phase sparse expert-parallel strategy:
  Phase A (gate): data-parallel over tokens (2048/core). fp32 gate matmul on
    PE + exact softmax -> gate_probs (bit-exact vs f32 reference on this HW).
  Host routing: top-2 selection from the returned probs (pure argsort-free
    masking, matches reference semantics), per-expert token gather, pad to
    capacity C, transpose to feature-major, cast to bf16.
  Phase B (FFN): expert-parallel - core e runs expert e's residual block over
    its C gathered tokens. Feature-major [H, tok] layout; LayerNorm over H via
    ones-matmul on PE; per-token stats replicated across partitions by PE
    outer products; combine weight applied on device; host scatter-adds the
    two contributions per token.
  Tokens beyond capacity (never expected for this distribution) are computed
  exactly on the host in f32.
"""

import sys

for _p in ("/opt/trn_rl_repo", "/root/.axon_site/_ro/trn_rl_repo"):
    if _p not in sys.path:
        sys.path.append(_p)

from contextlib import ExitStack

import ml_dtypes
import numpy as np

import concourse.bass as bass
import concourse.tile as tile
from concourse import bacc, mybir
from concourse.bass_interp import get_hw_module
from concourse.bass_utils import run_bass_kernel_spmd

F32 = mybir.dt.float32
BF16 = mybir.dt.bfloat16
AF = mybir.ActivationFunctionType
OP = mybir.AluOpType

N_CORES = 8
N, H, E = 16384, 1024, 8
NT = N // N_CORES          # tokens per core in gate phase (2048)
C = 4608                   # per-expert token capacity in FFN phase
TS = 512                   # token chunk (matmul free dim)
KT = H // 128              # 8 k/m tiles
LN_EPS = 1e-5

_CACHE = {}


def _build_gate():
    nc = bacc.Bacc("TRN2", target_bir_lowering=False, debug=False,
                   num_devices=N_CORES)
    xT = nc.dram_tensor("xT", [H, NT], F32, kind="ExternalInput").ap()
    wg = nc.dram_tensor("wg", [H, E], F32, kind="ExternalInput").ap()
    bgr = nc.dram_tensor("bgr", [128, E], F32, kind="ExternalInput").ap()
    gate = nc.dram_tensor("gate", [NT, E], F32, kind="ExternalOutput").ap()

    with tile.TileContext(nc) as tc, ExitStack() as ctx:
        const_pool = ctx.enter_context(tc.tile_pool(name="const", bufs=1))
        wg_sb = const_pool.tile([128, E * KT], F32, tag="wg")
        nc.sync.dma_start(wg_sb[:].rearrange("p (k e) -> p k e", k=KT),
                          wg.rearrange("(k p) e -> p k e", p=128))
        bg_sb = const_pool.tile([128, E], F32, tag="bg")
        nc.sync.dma_start(bg_sb[:], bgr)

        with tc.tile_pool(name="gx", bufs=KT) as gx_pool, \
             tc.tile_pool(name="gsb", bufs=3) as gsb, \
             tc.tile_pool(name="gps", bufs=8, space="PSUM") as gps:
            engines = [nc.sync, nc.gpsimd, nc.scalar]
            xks = []
            for k in range(KT):
                xk = gx_pool.tile([128, NT], F32, tag="xg")
                # two half-DMAs on different engines so slab loads spread
                # across queues instead of serializing on one
                h = NT // 2
                engines[k % 3].dma_start(xk[:, :h], xT[k * 128:(k + 1) * 128, :h])
                engines[(k + 1) % 3].dma_start(xk[:, h:],
                                               xT[k * 128:(k + 1) * 128, h:])
                xks.append(xk)
            NI = NT // 128
            gate3 = gate.rearrange("(i p) e -> p i e", p=128)
            for wave in range(NI // 8):
                wtiles = []
                for wi in range(8):
                    lg = gps.tile([128, E], F32, tag="lg")
                    wtiles.append(lg)
                for k in range(KT):
                    for wi in range(8):
                        i = wave * 8 + wi
                        nc.tensor.matmul(
                            wtiles[wi][:], xks[k][:, i * 128:(i + 1) * 128],
                            wg_sb[:, k * E:(k + 1) * E],
                            start=(k == 0), stop=(k == KT - 1))
                # batched softmax over the whole wave [128, 8 tiles x E]
                Lb = gsb.tile([128, 8 * E], F32, tag="L")
                for wi in range(8):
                    nc.vector.tensor_tensor(Lb[:, wi * E:(wi + 1) * E],
                                            wtiles[wi][:], bg_sb[:], OP.add)
                L3 = Lb[:].rearrange("p (i e) -> p i e", e=E)
                m1 = gsb.tile([128, 8], F32, tag="m1")
                nc.vector.tensor_reduce(m1[:], L3, axis=mybir.AxisListType.X,
                                        op=OP.max)
                m1b = m1[:].rearrange("p (i o) -> p i o", o=1).broadcast_to([128, 8, E])
                Ls = gsb.tile([128, 8 * E], F32, tag="Ls")
                nc.vector.tensor_tensor(
                    Ls[:].rearrange("p (i e) -> p i e", e=E), L3, m1b,
                    OP.subtract)
                ex = gsb.tile([128, 8 * E], F32, tag="ex")
                nc.scalar.activation(ex[:], Ls[:], AF.Exp)
                s = gsb.tile([128, 8], F32, tag="s")
                nc.vector.tensor_reduce(
                    s[:], ex[:].rearrange("p (i e) -> p i e", e=E),
                    axis=mybir.AxisListType.X, op=OP.add)
                sinv = gsb.tile([128, 8], F32, tag="sinv")
                nc.vector.reciprocal(sinv[:], s[:])
                sb = sinv[:].rearrange("p (i o) -> p i o", o=1).broadcast_to([128, 8, E])
                p = gsb.tile([128, 8 * E], F32, tag="p")
                nc.vector.tensor_tensor(
                    p[:].rearrange("p (i e) -> p i e", e=E),
                    ex[:].rearrange("p (i e) -> p i e", e=E), sb, OP.mult)
                nc.sync.dma_start(
                    gate3[:, wave * 8:(wave + 1) * 8, :],
                    p[:].rearrange("p (i e) -> p i e", e=E))

    nc.compile()
    nc.m = get_hw_module(nc.m)
    return nc


def _build_ffn():
    nc = bacc.Bacc("TRN2", target_bir_lowering=False, debug=False,
                   num_devices=N_CORES)
    xTe = nc.dram_tensor("xTe", [H, C], BF16, kind="ExternalInput").ap()
    w1e = nc.dram_tensor("w1e", [H, H], BF16, kind="ExternalInput").ap()
    w2e = nc.dram_tensor("w2e", [H, H], BF16, kind="ExternalInput").ap()
    wrow = nc.dram_tensor("wrow", [1, C], F32, kind="ExternalInput").ap()
    vec_in = {}
    for nm in ("b1", "g1", "be1", "b2", "g2", "be2"):
        vec_in[nm] = nc.dram_tensor(nm, [H], F32, kind="ExternalInput").ap()
    oute = nc.dram_tensor("oute", [H, C], F32, kind="ExternalOutput").ap()

    NCH = C // TS
    with tile.TileContext(nc) as tc, ExitStack() as ctx:
        const_pool = ctx.enter_context(tc.tile_pool(name="const", bufs=1))
        wt1 = const_pool.tile([128, KT * H], BF16, tag="wt1")
        wt2 = const_pool.tile([128, KT * H], BF16, tag="wt2")
        for k in range(KT):
            nc.sync.dma_start(wt1[:, k * H:(k + 1) * H],
                              w1e[k * 128:(k + 1) * 128, :])
        vecs = {}
        for nm in ("b1", "g1", "be1", "b2", "g2", "be2"):
            v = const_pool.tile([128, KT], F32, tag=f"v{nm}")
            nc.sync.dma_start(v[:], vec_in[nm].rearrange("(m p) -> p m", p=128))
            vecs[nm] = v
        ones_bf = const_pool.tile([128, 1], BF16, tag="ones_bf")
        nc.vector.memset(ones_bf[:], 1.0 / H)
        ones_orow = const_pool.tile([1, 128], BF16, tag="ones_orow")
        nc.vector.memset(ones_orow[:], 1.0)
        ones_orow32 = const_pool.tile([1, 128], F32, tag="ones_orow32")
        nc.vector.memset(ones_orow32[:], 1.0)
        eps_t = const_pool.tile([1, 1], F32, tag="eps")
        nc.vector.memset(eps_t[:], LN_EPS)

        with tc.tile_pool(name="xb", bufs=20) as xb_pool, \
             tc.tile_pool(name="hb", bufs=12) as hb_pool, \
             tc.tile_pool(name="sq", bufs=12) as sq_pool, \
             tc.tile_pool(name="tt", bufs=10) as t_pool, \
             tc.tile_pool(name="vf", bufs=10) as vf_pool, \
             tc.tile_pool(name="nt", bufs=6) as nt_pool, \
             tc.tile_pool(name="ab", bufs=4) as ab_pool, \
             tc.tile_pool(name="zz", bufs=8) as z_pool, \
             tc.tile_pool(name="row", bufs=8) as row_pool, \
             tc.tile_pool(name="wr", bufs=3) as wr_pool, \
             tc.tile_pool(name="mmp", bufs=5, space="PSUM") as mm_ps, \
             tc.tile_pool(name="mm2p", bufs=5, space="PSUM") as mm2_ps, \
             tc.tile_pool(name="stp", bufs=1, space="PSUM") as st_ps, \
             tc.tile_pool(name="rpp", bufs=2, space="PSUM") as rep_ps:

            def mm_phase(xin16, wt, pool, k_outer=False):
                pss = []
                if not k_outer:
                    for m in range(KT):
                        ps = pool.tile([128, TS], F32, tag="mm")
                        for k in range(KT):
                            nc.tensor.matmul(
                                ps[:],
                                wt[:, k * H + m * 128:k * H + (m + 1) * 128],
                                xin16[k][:], start=(k == 0),
                                stop=(k == KT - 1))
                        pss.append(ps)
                    return pss
                for g in range(2):
                    gps_tiles = []
                    for m in range(4):
                        ps = pool.tile([128, TS], F32, tag="mm")
                        gps_tiles.append(ps)
                    for k in range(KT):
                        for mi in range(4):
                            m = g * 4 + mi
                            nc.tensor.matmul(
                                gps_tiles[mi][:],
                                wt[:, k * H + m * 128:k * H + (m + 1) * 128],
                                xin16[k][:], start=(k == 0),
                                stop=(k == KT - 1))
                    pss.extend(gps_tiles)
                return pss

            def epilogue(pss, bias, gain, beta, store_bf16):
                hbs, sq_list = [], []
                for m in range(KT):
                    hb = hb_pool.tile([128, TS], BF16, tag="hb")
                    nc.scalar.activation(hb[:], pss[m][:], AF.Identity,
                                         bias=bias[:, m:m + 1], scale=1.0)
                    sq = sq_pool.tile([128, TS], BF16, tag="sq")
                    eng = nc.gpsimd if m % 2 == 0 else nc.vector
                    eng.tensor_tensor(sq[:], hb[:], hb[:], OP.mult)
                    hbs.append(hb)
                    sq_list.append(sq)
                s12 = st_ps.tile([33, TS], F32, tag="s1")
                s1 = s12[0:1, :]
                s2 = s12[32:33, :]
                for m in range(KT):
                    nc.tensor.matmul(s1, ones_bf[:], hbs[m][:],
                                     start=(m == 0), stop=(m == KT - 1))
                for m in range(KT):
                    nc.tensor.matmul(s2, ones_bf[:], sq_list[m][:],
                                     start=(m == 0), stop=(m == KT - 1))
                # mean path starts immediately after S1 stats (parallel
                # with the var->sqrt->recip chain)
                m16row = row_pool.tile([1, TS], BF16, tag="row16")
                nc.scalar.activation(m16row[:], s1, AF.Copy)
                m_ps = rep_ps.tile([128, TS], F32, tag="rep")
                nc.tensor.matmul(m_ps[:], ones_orow[:], m16row[:], start=True,
                                 stop=True)
                m16 = ab_pool.tile([128, TS], BF16, tag="ab")
                nc.scalar.activation(m16[:], m_ps[:], AF.Copy)
                s1sq = row_pool.tile([1, TS], F32, tag="row")
                nc.scalar.activation(s1sq[:], s1, AF.Square)
                var = row_pool.tile([1, TS], F32, tag="row")
                nc.vector.tensor_tensor(var[:], s2, s1sq[:], OP.subtract)
                stdv = row_pool.tile([1, TS], F32, tag="row")
                nc.scalar.activation(stdv[:], var[:], AF.Sqrt, bias=eps_t[:],
                                     scale=1.0)
                arow = row_pool.tile([1, TS], BF16, tag="row16")
                with nc.allow_low_precision(reason="rstd row replicated as bf16"):
                    nc.vector.reciprocal(arow[:], stdv[:])
                a_ps = rep_ps.tile([128, TS], F32, tag="rep")
                nc.tensor.matmul(a_ps[:], ones_orow[:], arow[:], start=True,
                                 stop=True)
                a16 = ab_pool.tile([128, TS], BF16, tag="ab")
                nc.scalar.activation(a16[:], a_ps[:], AF.Copy)

                outs = []
                for m in range(KT):
                    nt = nt_pool.tile([128, TS], BF16, tag="n")
                    nc.vector.tensor_tensor(nt[:], hbs[m][:], m16[:],
                                            OP.subtract)
                    nc.vector.tensor_tensor(nt[:], nt[:], a16[:], OP.mult)
                    if store_bf16:
                        o = t_pool.tile([128, TS], BF16, tag="t")
                    else:
                        o = vf_pool.tile([128, TS], BF16, tag="v")
                    nc.scalar.activation(
                        o[:], nt[:], AF.Relu if store_bf16 else AF.Identity,
                        bias=beta[:, m:m + 1], scale=gain[:, m:m + 1])
                    outs.append(o)
                return outs

            def load_x(c):
                tsl = slice(c * TS, (c + 1) * TS)
                x16 = []
                for k in range(KT):
                    xb = xb_pool.tile([128, TS], BF16, tag="x16")
                    nc.sync.dma_start(xb[:], xTe[k * 128:(k + 1) * 128, tsl])
                    x16.append(xb)
                return x16

            def combine(c, x16, vfs):
                tsl = slice(c * TS, (c + 1) * TS)
                wre = wr_pool.tile([1, TS], F32, tag="wre")
                nc.sync.dma_start(wre[:], wrow[:, tsl])
                w_ps = rep_ps.tile([128, TS], F32, tag="rep")
                nc.tensor.matmul(w_ps[:], ones_orow32[:], wre[:], start=True,
                                 stop=True)
                for m in range(KT):
                    v = vfs[m]
                    nc.vector.tensor_tensor(v[:], v[:], x16[m][:], OP.add)
                    z = z_pool.tile([128, TS], F32, tag="z")
                    nc.vector.scalar_tensor_tensor(z[:], v[:], 0.0, w_ps[:],
                                                   OP.max, OP.mult)
                    nc.sync.dma_start(oute[m * 128:(m + 1) * 128, tsl], z[:])

            for c in range(NCH):
                x16 = load_x(c)
                ps1 = mm_phase(x16, wt1, mm_ps)
                if c == 0:
                    # deferred so chunk-0 x DMAs aren't queued behind 2MB of
                    # layer-2 weights nobody needs for ~30us
                    for k in range(KT):
                        nc.sync.dma_start(wt2[:, k * H:(k + 1) * H],
                                          w2e[k * 128:(k + 1) * 128, :])
                t16 = epilogue(ps1, vecs["b1"], vecs["g1"], vecs["be1"], True)
                ps2 = mm_phase(t16, wt2, mm_ps)
                vfs = epilogue(ps2, vecs["b2"], vecs["g2"], vecs["be2"], False)
                combine(c, x16, vfs)

    nc.compile()
    nc.m = get_hw_module(nc.m)
    return nc


def _ffn_host_exact(x_tok, W1e, b1e, g1e, be1e, W2e, b2e, g2e, be2e):
    """Exact f32 single-expert residual block for overflow tokens."""
    h = x_tok @ W1e + b1e
    mu = h.mean(-1, keepdims=True)
    va = ((h - mu) ** 2).mean(-1, keepdims=True)
    t = np.maximum((h - mu) / np.sqrt(va + LN_EPS) * g1e + be1e, 0)
    h2 = t @ W2e + b2e
    mu2 = h2.mean(-1, keepdims=True)
    va2 = ((h2 - mu2) ** 2).mean(-1, keepdims=True)
    v = (h2 - mu2) / np.sqrt(va2 + LN_EPS) * g2e + be2e
    return np.maximum(x_tok + v, 0)


def kernel(x, Wg, bg, W1, b1, g1, be1, W2, b2, g2, be2,
           _trace=False, _tmpdir=None):
    x = np.ascontiguousarray(np.asarray(x, np.float32))
    Wg = np.asarray(Wg, np.float32)
    bg = np.asarray(bg, np.float32)
    W1 = np.asarray(W1, np.float32)
    W2 = np.asarray(W2, np.float32)
    vec = {nm: np.ascontiguousarray(np.asarray(v, np.float32))
           for nm, v in (("b1", b1), ("g1", g1), ("be1", be1),
                         ("b2", b2), ("g2", g2), ("be2", be2))}

    # ---- phase A: gate on device ----
    if "gate" not in _CACHE:
        _CACHE["gate"] = _build_gate()
    gshared = {
        "wg": np.ascontiguousarray(Wg),
        "bgr": np.ascontiguousarray(np.broadcast_to(bg[None, :], (128, E))),
    }
    gmaps = []
    for cix in range(N_CORES):
        m = dict(gshared)
        m["xT"] = np.ascontiguousarray(x[cix * NT:(cix + 1) * NT].T)
        gmaps.append(m)
    gres = run_bass_kernel_spmd(_CACHE["gate"], gmaps, list(range(N_CORES)))
    gate = np.concatenate([gres.results[cix]["gate"]
                           for cix in range(N_CORES)], axis=0)

    # ---- host routing (selection identical to reference's top_k on probs) ----
    p = gate
    m1 = p.max(1, keepdims=True)
    pmask = np.where(p >= m1, -np.inf, p)
    m2 = pmask.max(1, keepdims=True).astype(np.float32)
    topmask = p >= m2                                   # [N, E] top-2 mask
    w = (p * topmask / (m1 + m2 + 1e-9)).astype(np.float32)

    W1b = W1.astype(ml_dtypes.bfloat16)
    W2b = W2.astype(ml_dtypes.bfloat16)
    xb16 = x.astype(ml_dtypes.bfloat16)

    idx_list, over_list = [], []
    fmaps = []
    for e in range(E):
        idx = np.nonzero(topmask[:, e])[0]
        over = idx[C:] if len(idx) > C else idx[:0]
        idx = idx[:C]
        idx_list.append(idx)
        over_list.append(over)
        xe = np.zeros((C, H), ml_dtypes.bfloat16)
        xe[:len(idx)] = xb16[idx]
        wr = np.zeros((1, C), np.float32)
        wr[0, :len(idx)] = w[idx, e]
        fmaps.append({
            "xTe": np.ascontiguousarray(xe.T),
            "w1e": np.ascontiguousarray(W1b[e]),
            "w2e": np.ascontiguousarray(W2b[e]),
            "wrow": wr,
            "b1": vec["b1"][e], "g1": vec["g1"][e], "be1": vec["be1"][e],
            "b2": vec["b2"][e], "g2": vec["g2"][e], "be2": vec["be2"][e],
        })

    # ---- phase B: expert-parallel FFN on device ----
    if "ffn" not in _CACHE:
        _CACHE["ffn"] = _build_ffn()
    fres = run_bass_kernel_spmd(_CACHE["ffn"], fmaps, list(range(N_CORES)),
                                trace=_trace, tmpdir=_tmpdir)
    _CACHE["last"] = fres

    moe = np.zeros((N, H), np.float32)
    for e in range(E):
        idx = idx_list[e]
        oe = fres.results[e]["oute"]                    # [H, C] f32
        moe[idx] += oe[:, :len(idx)].T
        if len(over_list[e]):
            ov = over_list[e]
            y = _ffn_host_exact(x[ov], W1[e], vec["b1"][e], vec["g1"][e],
                                vec["be1"][e], W2[e], vec["b2"][e],
                                vec["g2"][e], vec["be2"][e])
            moe[ov] += w[ov, e][:, None] * y
    return moe, gate
